# revision 10
# baseline (speedup 1.0000x reference)
"""Fused NMS-detection kernel for Trainium2 (8 NeuronCores, Bass/Tile).

Matches reference.py:
  patchify -> feats = patches @ W -> sim = feats @ feats.T -> degree counts
  -> seed = argmin(counts) -> top-K(sim[seed]) keep mask -> sequential
  region-growing scan -> [64, 64] binary mask.

The 4096x4096 sim matrix is never materialized to HBM: column counts of
(sim >= 0) are reduced tile-by-tile out of PSUM, and the sequential scan
runs on a compacted 112x112 Gram matrix M = F_A @ F_A^T over the <=100
kept positions (recurrence s_k = base_k + M[k, :] . delta, where delta
tracks 1->0 flips in scan order).

Sharding: patch columns split 512/core; each core computes its F^T slice;
6 chunked AllGathers assemble the full F^T (12.6MB) on every core; the
count phase is j-sliced per core; after the (tiny) argmin AllGather the
cheap tail phases run replicated on all cores.
"""
import sys

sys.path.insert(0, "/opt/trn_rl_repo")

from contextlib import ExitStack

import numpy as np

import concourse.bass as bass
import concourse.bacc as bacc
import concourse.mybir as mybir
import concourse.tile as tile
from concourse.bass_utils import run_bass_kernel_spmd

dt = mybir.dt
Alu = mybir.AluOpType
AX = mybir.AxisListType

P = 16          # patch size
D = 768         # embed dim
N = 4096        # num patches (64x64)
K = 100         # top-K kept
NCORES = 8
JSL = N // NCORES   # 512 j-columns per core
NCH = D // 128      # 6 contraction chunks
KPAD = 112          # kept list padded to 16*7
NEG = -1.0e30


def build_program():
    nc = bacc.Bacc(None, "TRN2", target_bir_lowering=False, num_devices=NCORES)

    pTs = nc.dram_tensor("pTs", [NCH, 128, JSL], dt.float32, kind="ExternalInput")
    Wp = nc.dram_tensor("Wp", [NCH, 128, D], dt.float32, kind="ExternalInput")
    jbase = nc.dram_tensor("jbase", [1, 1], dt.float32, kind="ExternalInput")
    cmult = nc.dram_tensor("cmult", [128, 1], dt.float32, kind="ExternalInput")
    out = nc.dram_tensor("out", [N], dt.float32, kind="ExternalOutput")

    with tile.TileContext(nc) as tc, ExitStack() as ctx:
        big = ctx.enter_context(tc.tile_pool(name="big", bufs=1))
        sml = ctx.enter_context(tc.tile_pool(name="sml", bufs=1))
        ps = ctx.enter_context(tc.tile_pool(name="ps", bufs=4, space="PSUM"))
        ps1 = ctx.enter_context(tc.tile_pool(name="ps1", bufs=1, space="PSUM"))
        dram = ctx.enter_context(tc.tile_pool(name="dram", bufs=1, space="DRAM"))

        # ---- load inputs -------------------------------------------------
        t_pTs = big.tile([128, NCH, JSL], dt.float32, tag="pTs")
        for c in range(NCH):
            nc.sync.dma_start(t_pTs[:, c, :], pTs[c, :, :])
        t_Wp = big.tile([128, NCH, D], dt.float32, tag="Wp")
        for c in range(NCH):
            nc.sync.dma_start(t_Wp[:, c, :], Wp[c, :, :])
        t_jbase = sml.tile([1, 1], dt.float32)
        nc.sync.dma_start(t_jbase[:], jbase[:])
        t_cmult = sml.tile([128, 1], dt.float32)
        nc.sync.dma_start(t_cmult[:], cmult[:])

        # ---- P1: myJ = F^T[:, my j-slice]  (6 d-chunks x [128, 512]) -----
        myJ = big.tile([128, NCH, JSL], dt.float32, tag="myJ")
        for db in range(NCH):
            pacc = ps.tile([128, JSL], dt.float32, tag="pbig")
            for c in range(NCH):
                nc.tensor.matmul(
                    pacc[:],
                    t_Wp[:, c, db * 128:(db + 1) * 128],
                    t_pTs[:, c, :],
                    start=(c == 0),
                    stop=(c == NCH - 1),
                )
            nc.scalar.copy(myJ[:, db, :], pacc[:])

        # ---- P2: AllGather the full F^T ----------------------------------
        FT = big.tile([128, NCH, N], dt.float32, tag="FT")
        for c in range(NCH):
            agin = dram.tile([128, JSL], dt.float32, tag=f"agin{c}")
            agout = dram.tile([NCORES, 128, JSL], dt.float32, tag=f"agout{c}")
            nc.sync.dma_start(agin[:], myJ[:, c, :])
            nc.gpsimd.collective_compute(
                "AllGather",
                Alu.bypass,
                replica_groups=[list(range(NCORES))],
                ins=[agin[:].opt()],
                outs=[agout[:].opt()],
            )
            # [g, p, j'] -> FT[p, c, g*512 + j']
            for g in range(NCORES):
                nc.sync.dma_start(FT[:, c, g * JSL:(g + 1) * JSL], agout[g, :, :])

        # ---- P3: counts of (sim >= 0) for my j-slice ---------------------
        NIT = N // 512   # 8 i-tiles
        NJB = JSL // 128  # 4 j-blocks
        acc = sml.tile([128, NJB * NIT], dt.float32)
        for jb in range(NJB):
            for it in range(NIT):
                psim = ps.tile([128, 512], dt.float32, tag="pbig")
                for c in range(NCH):
                    nc.tensor.matmul(
                        psim[:],
                        myJ[:, c, jb * 128:(jb + 1) * 128],
                        FT[:, c, it * 512:(it + 1) * 512],
                        start=(c == 0),
                        stop=(c == NCH - 1),
                    )
                nc.vector.tensor_scalar(
                    psim[:], psim[:], 0.0, None, Alu.is_ge, Alu.add,
                    accum_out=acc[:, jb * NIT + it:jb * NIT + it + 1],
                )
        counts = sml.tile([128, NJB], dt.float32)
        for jb in range(NJB):
            nc.vector.tensor_reduce(
                counts[:, jb:jb + 1], acc[:, jb * NIT:(jb + 1) * NIT], AX.X, Alu.add)

        # ---- P4: global argmin seed --------------------------------------
        # negkey = -(count*4096 + jglobal); maximize
        jb_b = sml.tile([128, 1], dt.float32)
        nc.gpsimd.partition_broadcast(jb_b[:], t_jbase[:], channels=128)
        jloc = sml.tile([128, NJB], dt.int32)
        nc.gpsimd.iota(jloc[:], pattern=[[128, NJB]], base=0, channel_multiplier=1)
        jlocf = sml.tile([128, NJB], dt.float32)
        nc.vector.tensor_copy(jlocf[:], jloc[:])
        jglob = sml.tile([128, NJB], dt.float32)
        nc.vector.tensor_scalar(jglob[:], jlocf[:], jb_b[:], None, Alu.add)
        negkey = sml.tile([128, NJB], dt.float32)
        nc.vector.scalar_tensor_tensor(
            negkey[:], counts[:], -4096.0, jglob[:], Alu.mult, Alu.subtract)
        nk1 = sml.tile([128, 1], dt.float32)
        nc.vector.tensor_reduce(nk1[:], negkey[:], AX.X, Alu.max)
        mykey = sml.tile([1, 1], dt.float32)
        nc.gpsimd.tensor_reduce(mykey[:], nk1[:], AX.C, Alu.max)
        # pad the collective payload to 512B
        zk = sml.tile([1, 128], dt.float32)
        nc.gpsimd.memset(zk[:], 0.0)
        myk128 = sml.tile([1, 128], dt.float32)
        nc.vector.tensor_scalar(myk128[:], zk[:], mykey[:], None, Alu.add)
        kin = dram.tile([1, 128], dt.float32, tag="kin")
        kout = dram.tile([NCORES, 128], dt.float32, tag="kout")
        nc.sync.dma_start(kin[:], myk128[:])
        nc.gpsimd.collective_compute(
            "AllGather",
            Alu.bypass,
            replica_groups=[list(range(NCORES))],
            ins=[kin[:].opt()],
            outs=[kout[:].opt()],
        )
        kall = sml.tile([1, NCORES], dt.float32)
        nc.sync.dma_start(kall[:], kout[:, 0:1])
        gnk = sml.tile([1, 1], dt.float32)
        nc.vector.tensor_reduce(gnk[:], kall[:], AX.X, Alu.max)
        gkey = sml.tile([1, 1], dt.float32)
        nc.vector.tensor_scalar(gkey[:], gnk[:], -1.0, None, Alu.mult)
        gkey_i = sml.tile([1, 1], dt.int32)
        nc.vector.tensor_copy(gkey_i[:], gkey[:])
        seed_i = sml.tile([1, 1], dt.int32)
        nc.vector.tensor_scalar(seed_i[:], gkey_i[:], 4095, None, Alu.bitwise_and)
        seed_f = sml.tile([1, 1], dt.float32)
        nc.vector.tensor_copy(seed_f[:], seed_i[:])

        # ---- P5: sim_seed in column layout [128, 32] ---------------------
        seed_b = sml.tile([128, 1], dt.float32)
        nc.gpsimd.partition_broadcast(seed_b[:], seed_f[:], channels=128)
        fsidxf = sml.tile([128, 1], dt.float32)
        nc.vector.tensor_scalar(fsidxf[:], t_cmult[:], seed_b[:], None, Alu.add)
        fsidx = sml.tile([128, 1], dt.int16)
        nc.vector.tensor_copy(fsidx[:], fsidxf[:])
        fs16 = sml.tile([128, 16], dt.float32)
        FT_flat = FT[:].rearrange("p a b -> p (a b)")
        nc.gpsimd.ap_gather(
            fs16[:], FT_flat, fsidx[:],
            channels=128, num_elems=NCH * N, d=1, num_idxs=16,
        )
        pss = ps1.tile([128, 32], dt.float32, tag="pss")
        for cb in range(32):
            for c in range(NCH):
                nc.tensor.matmul(
                    pss[:, cb:cb + 1],
                    FT[:, c, cb * 128:(cb + 1) * 128],
                    fs16[:, c:c + 1],
                    start=(c == 0),
                    stop=(c == NCH - 1),
                )
        ss_col = sml.tile([128, 32], dt.float32)
        nc.vector.tensor_copy(ss_col[:], pss[:])

        # ---- P6: top-100 threshold T -------------------------------------
        r1 = sml.tile([128, 8], dt.float32)
        nc.vector.max(r1[:], ss_col[:])
        ssr = sml.tile([128, 32], dt.float32)
        nc.vector.match_replace(ssr[:], r1[:], ss_col[:], NEG)
        cand = sml.tile([128, 16], dt.float32)
        nc.vector.tensor_copy(cand[:, 0:8], r1[:])
        nc.vector.max(cand[:, 8:16], ssr[:])
        # regroup to [16, 128]: cgrp[p16, g*16+r] = cand[g*16+p16, r]
        cgrp = sml.tile([16, 128], dt.float32)
        for g in range(8):
            nc.sync.dma_start(
                cgrp[:, g * 16:(g + 1) * 16], cand[g * 16:(g + 1) * 16, :])
        pool24 = sml.tile([16, 24], dt.float32)
        nc.vector.max(pool24[:, 0:8], cgrp[:])
        cg2 = sml.tile([16, 128], dt.float32)
        nc.vector.match_replace(cg2[:], pool24[:, 0:8], cgrp[:], NEG)
        nc.vector.max(pool24[:, 8:16], cg2[:])
        cg3 = sml.tile([16, 128], dt.float32)
        nc.vector.match_replace(cg3[:], pool24[:, 8:16], cg2[:], NEG)
        nc.vector.max(pool24[:, 16:24], cg3[:])
        wrk0 = sml.tile([1, 384], dt.float32, tag="wrkA")
        nc.sync.dma_start(
            wrk0[:].rearrange("o (p f) -> o p f", p=16, f=24), pool24[:])
        m8 = sml.tile([1, 8], dt.float32)
        wrk = wrk0
        for r in range(13):
            nc.vector.max(m8[:], wrk[:])
            if r < 12:
                wrk2 = sml.tile([1, 384], dt.float32, tag=f"wrk{'B' if r % 2 == 0 else 'A'}")
                nc.vector.match_replace(wrk2[:], m8[:], wrk[:], NEG)
                wrk = wrk2
        Tthr = sml.tile([1, 1], dt.float32)
        nc.vector.tensor_copy(Tthr[:], m8[:, 3:4])

        # ---- P7: keep mask + compaction ----------------------------------
        # ss16[p16, f] = sim_seed[f*16 + p16]; j = c*128 + g*16 + p16,
        # f = j // 16 = c*8 + g
        ss16 = sml.tile([16, 256], dt.float32)
        for g in range(8):
            src = ss_col[g * 16:(g + 1) * 16, :]
            dst = ss16[:].rearrange("p (c g) -> p c g", c=32, g=8)[:, :, g]
            nc.sync.dma_start(dst, src)
        T16 = sml.tile([16, 1], dt.float32)
        nc.gpsimd.partition_broadcast(T16[:], Tthr[:], channels=16)
        keep16 = sml.tile([16, 256], dt.float32)
        nc.vector.tensor_scalar(keep16[:], ss16[:], T16[:], None, Alu.is_ge)
        jv1 = sml.tile([16, 256], dt.float32)
        nc.gpsimd.iota(jv1[:], pattern=[[16, 256]], base=1, channel_multiplier=1,
                       allow_small_or_imprecise_dtypes=True)
        arr1 = sml.tile([16, 256], dt.float32)
        nc.vector.tensor_tensor(arr1[:], keep16[:], jv1[:], Alu.mult)
        nc.vector.tensor_scalar(arr1[:], arr1[:], -1.0, None, Alu.add)
        ge0 = sml.tile([16, 256], dt.float32)
        nc.vector.tensor_scalar(ge0[:], ss16[:], 0.0, None, Alu.is_ge)
        arr2 = sml.tile([16, 256], dt.float32)
        nc.vector.scalar_tensor_tensor(
            arr2[:], ge0[:], 2.0, keep16[:], Alu.add, Alu.mult)
        nc.vector.tensor_scalar(arr2[:], arr2[:], -1.0, None, Alu.add)
        idxf16 = sml.tile([16, 8], dt.float32)
        curf16 = sml.tile([16, 8], dt.float32)
        nf1 = sml.tile([1, 1], dt.uint32)
        nf2 = sml.tile([1, 1], dt.uint32)
        nc.gpsimd.sparse_gather(idxf16[:], arr1[:], num_found=nf1[:])
        nc.gpsimd.sparse_gather(curf16[:], arr2[:], num_found=nf2[:])
        # clean pads (l >= 100): idx -> -1, cur01 -> 0
        lg = sml.tile([16, 8], dt.int32)
        nc.gpsimd.iota(lg[:], pattern=[[16, 8]], base=0, channel_multiplier=1)
        lgf = sml.tile([16, 8], dt.float32)
        nc.vector.tensor_copy(lgf[:], lg[:])
        validl = sml.tile([16, 8], dt.float32)
        nc.vector.tensor_scalar(validl[:], lgf[:], float(K) - 0.5, None, Alu.is_le)
        idxc16 = sml.tile([16, 8], dt.float32)
        nc.vector.scalar_tensor_tensor(
            idxc16[:], idxf16[:], 1.0, validl[:], Alu.add, Alu.mult)
        nc.vector.tensor_scalar(idxc16[:], idxc16[:], -1.0, None, Alu.add)
        cur16 = sml.tile([16, 8], dt.float32)
        nc.vector.scalar_tensor_tensor(
            cur16[:], curf16[:], -1.0, validl[:], Alu.add, Alu.mult)

        # ---- P8: gather F_A^T chunks; M; base ----------------------------
        # idx_row / cur_row [1, 112] in l = f*16+p order
        idx_row = sml.tile([1, KPAD], dt.float32)
        cur_row = sml.tile([1, KPAD], dt.float32)
        for f in range(7):
            nc.sync.dma_start(idx_row[0:1, f * 16:(f + 1) * 16], idxc16[:, f:f + 1])
            nc.sync.dma_start(cur_row[0:1, f * 16:(f + 1) * 16], cur16[:, f:f + 1])
        # wrapped idx for ap_gather: replicate [16, 7] -> [128, 7]
        idx_wr = sml.tile([128, 7], dt.float32)
        for g in range(8):
            nc.sync.dma_start(idx_wr[g * 16:(g + 1) * 16, :], idxc16[:, 0:7])
        nc.vector.tensor_scalar_max(idx_wr[:], idx_wr[:], 0.0)
        FA_T = sml.tile([128, NCH, KPAD], dt.float32)
        nc.gpsimd.memset(FA_T[:], 0.0)
        for c in range(NCH):
            idx_wf = sml.tile([128, 7], dt.float32, tag="idxwf")
            nc.vector.tensor_scalar(
                idx_wf[:], idx_wr[:], float(c * N), None, Alu.add)
            idx_wi = sml.tile([128, 7], dt.int16, tag="idxwi")
            nc.vector.tensor_copy(idx_wi[:], idx_wf[:])
            nc.gpsimd.ap_gather(
                FA_T[:, c, :], FT_flat, idx_wi[:],
                channels=128, num_elems=NCH * N, d=1, num_idxs=KPAD,
            )
        pM = ps1.tile([KPAD, KPAD], dt.float32, tag="pM")
        for c in range(NCH):
            nc.tensor.matmul(
                pM[:], FA_T[:, c, :], FA_T[:, c, :],
                start=(c == 0), stop=(c == NCH - 1),
            )
        M_sb = sml.tile([KPAD, KPAD], dt.float32)
        nc.vector.tensor_copy(M_sb[:], pM[:])
        M_flat = sml.tile([1, KPAD * KPAD], dt.float32)
        nc.sync.dma_start(
            M_flat[:].rearrange("o (p f) -> o p f", p=KPAD, f=KPAD), M_sb[:])
        cur_col = sml.tile([KPAD, 1], dt.float32)
        nc.sync.dma_start(cur_col[:], cur_row[0:1, :])
        pbase = ps1.tile([KPAD, 1], dt.float32, tag="pbase")
        nc.tensor.matmul(pbase[:], M_sb[:], cur_col[:], start=True, stop=True)
        bcol = sml.tile([KPAD, 1], dt.float32)
        nc.vector.tensor_copy(bcol[:], pbase[:])
        base_row = sml.tile([1, KPAD], dt.float32)
        nc.sync.dma_start(base_row[0:1, :], bcol[:, 0:1])

        # ---- sequential region-growing over the 100 kept positions -------
        delta = sml.tile([1, KPAD], dt.float32)
        nc.gpsimd.memset(delta[:], 0.0)
        zero1 = sml.tile([1, 1], dt.float32)
        nc.gpsimd.memset(zero1[:], 0.0)
        prod = sml.tile([1, KPAD], dt.float32)
        s1 = sml.tile([1, 1], dt.float32)
        u1 = sml.tile([1, 1], dt.float32)
        for k in range(K):
            nc.vector.scalar_tensor_tensor(
                prod[:], M_flat[0:1, k * KPAD:(k + 1) * KPAD], 0.0, delta[:],
                Alu.bypass, Alu.mult, accum_out=s1[:])
            nc.vector.scalar_tensor_tensor(
                u1[:], s1[:], base_row[0:1, k:k + 1], zero1[:],
                Alu.add, Alu.is_gt)
            nc.vector.scalar_tensor_tensor(
                delta[0:1, k:k + 1], u1[:], cur_row[0:1, k:k + 1],
                cur_row[0:1, k:k + 1], Alu.mult, Alu.subtract)
        act_row = sml.tile([1, KPAD], dt.float32)
        nc.vector.tensor_tensor(act_row[:], cur_row[:], delta[:], Alu.add)

        # ---- P9: scatter back via one-hot matmul -------------------------
        act_col = sml.tile([KPAD, 1], dt.float32)
        nc.sync.dma_start(act_col[:], act_row[0:1, :])
        idx_col = sml.tile([KPAD, 1], dt.float32)
        nc.sync.dma_start(idx_col[:], idx_row[0:1, :])
        jgrid = big.tile([128, N], dt.float32, tag="FT")
        nc.gpsimd.iota(jgrid[:], pattern=[[1, N]], base=0, channel_multiplier=0,
                       allow_small_or_imprecise_dtypes=True)
        E = big.tile([KPAD, N], dt.float32, tag="Wp")
        nc.vector.tensor_scalar(
            E[:], jgrid[0:KPAD, :], idx_col[:], None, Alu.is_equal)
        pout = ps1.tile([128, 32], dt.float32, tag="pss")
        for cb in range(32):
            nc.tensor.matmul(
                pout[:, cb:cb + 1], E[:, cb * 128:(cb + 1) * 128], act_col[:],
                start=True, stop=True)
        outcol = sml.tile([128, 32], dt.float32)
        nc.vector.tensor_copy(outcol[:], pout[:])
        nc.sync.dma_start(
            out[:].rearrange("(c p) -> p c", p=128, c=32), outcol[:])

    nc.compile()
    return nc


_NC_CACHE = None


def _get_nc():
    global _NC_CACHE
    if _NC_CACHE is None:
        _NC_CACHE = build_program()
    return _NC_CACHE


def make_in_maps(img: np.ndarray, W_patch: np.ndarray):
    img = np.asarray(img, np.float32)
    W_patch = np.asarray(W_patch, np.float32)
    x = img[0].reshape(3, 64, P, 64, P).transpose(1, 3, 2, 4, 0)
    patches = np.ascontiguousarray(x).reshape(N, P * P * 3)
    pT = np.ascontiguousarray(patches.T)          # [768, 4096]
    pT6 = pT.reshape(NCH, 128, N)
    Wp = np.ascontiguousarray(W_patch.reshape(NCH, 128, D))
    pvals = np.arange(128) % 16
    cmult = np.where(pvals < NCH, pvals * float(N), 0.0).astype(np.float32)
    cmult = cmult.reshape(128, 1)
    in_maps = []
    for c in range(NCORES):
        in_maps.append({
            "pTs": np.ascontiguousarray(pT6[:, :, c * JSL:(c + 1) * JSL]),
            "Wp": Wp,
            "jbase": np.array([[c * JSL]], np.float32),
            "cmult": cmult,
        })
    return in_maps


def kernel(img: np.ndarray, W_patch: np.ndarray) -> np.ndarray:
    nc = _get_nc()
    in_maps = make_in_maps(img, W_patch)
    res = run_bass_kernel_spmd(nc, in_maps, core_ids=list(range(NCORES)))
    return res.results[0]["out"].reshape(64, 64).astype(np.float32)


# revision 13
# speedup vs baseline: 1.0089x; 1.0089x over previous
"""Fused NMS-detection kernel for Trainium2 (8 NeuronCores, Bass/Tile).

Matches reference.py:
  patchify -> feats = patches @ W -> sim = feats @ feats.T -> degree counts
  -> seed = argmin(counts) -> top-K(sim[seed]) keep mask -> sequential
  region-growing scan -> [64, 64] binary mask.

The 4096x4096 sim matrix is never materialized to HBM: column counts of
(sim >= 0) are reduced tile-by-tile out of PSUM, and the sequential scan
runs on a compacted 112x112 Gram matrix M = F_A @ F_A^T over the <=100
kept positions (recurrence s_k = M[k, :] . state, where state holds
updated values for processed positions and initial values otherwise).

Sharding: patch columns split 512/core; each core computes its F^T slice;
6 chunked AllGathers assemble the full F^T (12.6MB) on every core; the
count phase is j-sliced per core; after the (tiny) argmin AllGather the
cheap tail phases run replicated on all cores.
"""
import sys

sys.path.insert(0, "/opt/trn_rl_repo")

from contextlib import ExitStack

import numpy as np

import concourse.bass as bass
import concourse.bacc as bacc
import concourse.bass_isa as bass_isa
import concourse.mybir as mybir
import concourse.tile as tile
from concourse.bass_utils import run_bass_kernel_spmd

dt = mybir.dt
Alu = mybir.AluOpType
AX = mybir.AxisListType

P = 16          # patch size
D = 768         # embed dim
N = 4096        # num patches (64x64)
K = 100         # top-K kept
NCORES = 8
JSL = N // NCORES   # 512 j-columns per core
NCH = D // 128      # 6 contraction chunks
KPAD = 112          # kept list padded to 16*7
NEG = -1.0e30


def build_program():
    nc = bacc.Bacc(None, "TRN2", target_bir_lowering=False, num_devices=NCORES)

    pTs = nc.dram_tensor("pTs", [NCH, 128, JSL], dt.float32, kind="ExternalInput")
    Wp = nc.dram_tensor("Wp", [NCH, 128, D], dt.float32, kind="ExternalInput")
    jbase = nc.dram_tensor("jbase", [1, 1], dt.float32, kind="ExternalInput")
    cmult = nc.dram_tensor("cmult", [128, 1], dt.float32, kind="ExternalInput")
    out = nc.dram_tensor("out", [N], dt.float32, kind="ExternalOutput")

    with tile.TileContext(nc) as tc, ExitStack() as ctx:
        big = ctx.enter_context(tc.tile_pool(name="big", bufs=1))
        sml = ctx.enter_context(tc.tile_pool(name="sml", bufs=1))
        ps = ctx.enter_context(tc.tile_pool(name="ps", bufs=4, space="PSUM"))
        ps1 = ctx.enter_context(tc.tile_pool(name="ps1", bufs=1, space="PSUM"))
        dram = ctx.enter_context(tc.tile_pool(name="dram", bufs=1, space="DRAM"))

        # ---- load inputs -------------------------------------------------
        t_pTs = big.tile([128, NCH, JSL], dt.float32, tag="pTs")
        for c in range(NCH):
            nc.sync.dma_start(t_pTs[:, c, :], pTs[c, :, :])
        t_Wp = big.tile([128, NCH, D], dt.float32, tag="Wp")
        for c in range(NCH):
            nc.sync.dma_start(t_Wp[:, c, :], Wp[c, :, :])
        t_jbase = sml.tile([1, 1], dt.float32)
        nc.sync.dma_start(t_jbase[:], jbase[:])
        t_cmult = sml.tile([128, 1], dt.float32)
        nc.sync.dma_start(t_cmult[:], cmult[:])

        # ---- P1: myJ = F^T[:, my j-slice]  (6 d-chunks x [128, 512]) -----
        myJ = big.tile([128, NCH, JSL], dt.float32, tag="myJ")
        for db in range(NCH):
            pacc = ps.tile([128, JSL], dt.float32, tag="pbig")
            for c in range(NCH):
                nc.tensor.matmul(
                    pacc[:],
                    t_Wp[:, c, db * 128:(db + 1) * 128],
                    t_pTs[:, c, :],
                    start=(c == 0),
                    stop=(c == NCH - 1),
                )
            nc.scalar.copy(myJ[:, db, :], pacc[:])

        # ---- P2: AllGather the full F^T (Shared outputs) -----------------
        FT = big.tile([128, NCH, N], dt.float32, tag="FT")
        agouts = [
            nc.dram_tensor(f"agout{c}", [NCORES, 128, JSL], dt.float32,
                           addr_space="Shared")
            for c in range(NCH)
        ]
        for c in range(NCH):
            agin = dram.tile([128, JSL], dt.float32, tag=f"agin{c}")
            nc.sync.dma_start(agin[:], myJ[:, c, :])
            nc.gpsimd.collective_compute(
                "AllGather",
                Alu.bypass,
                replica_groups=[list(range(NCORES))],
                ins=[agin[:].opt()],
                outs=[agouts[c][:].opt()],
            )
            # [g, p, j'] -> FT[p, c, g*512 + j']
            for g in range(NCORES):
                nc.sync.dma_start(
                    FT[:, c, g * JSL:(g + 1) * JSL], agouts[c][g, :, :])

        # ---- P3: counts of (sim >= 0) for my j-slice ---------------------
        NIT = N // 512   # 8 i-tiles
        NJB = JSL // 128  # 4 j-blocks
        acc = sml.tile([128, NJB * NIT], dt.float32)
        for jb in range(NJB):
            for it in range(NIT):
                psim = ps.tile([128, 512], dt.float32, tag="pbig")
                for c in range(NCH):
                    nc.tensor.matmul(
                        psim[:],
                        myJ[:, c, jb * 128:(jb + 1) * 128],
                        FT[:, c, it * 512:(it + 1) * 512],
                        start=(c == 0),
                        stop=(c == NCH - 1),
                    )
                nc.vector.tensor_scalar(
                    psim[:], psim[:], 0.0, None, Alu.is_ge, Alu.add,
                    accum_out=acc[:, jb * NIT + it:jb * NIT + it + 1],
                )
        counts = sml.tile([128, NJB], dt.float32)
        for jb in range(NJB):
            nc.vector.tensor_reduce(
                counts[:, jb:jb + 1], acc[:, jb * NIT:(jb + 1) * NIT], AX.X, Alu.add)

        # ---- P4: global argmin seed --------------------------------------
        # negkey = -(count*4096 + jglobal); maximize
        jb_b = sml.tile([128, 1], dt.float32)
        nc.gpsimd.partition_broadcast(jb_b[:], t_jbase[:], channels=128)
        jloc = sml.tile([128, NJB], dt.int32)
        nc.gpsimd.iota(jloc[:], pattern=[[128, NJB]], base=0, channel_multiplier=1)
        jlocf = sml.tile([128, NJB], dt.float32)
        nc.vector.tensor_copy(jlocf[:], jloc[:])
        jglob = sml.tile([128, NJB], dt.float32)
        nc.vector.tensor_scalar(jglob[:], jlocf[:], jb_b[:], None, Alu.add)
        negkey = sml.tile([128, NJB], dt.float32)
        nc.vector.scalar_tensor_tensor(
            negkey[:], counts[:], -4096.0, jglob[:], Alu.mult, Alu.subtract)
        nk1 = sml.tile([128, 1], dt.float32)
        nc.vector.tensor_reduce(nk1[:], negkey[:], AX.X, Alu.max)
        nkar = sml.tile([128, 1], dt.float32)
        nc.gpsimd.partition_all_reduce(
            nkar[:], nk1[:], channels=128, reduce_op=bass_isa.ReduceOp.max)
        # pad the collective payload to 512B
        zk = sml.tile([1, 128], dt.float32)
        nc.vector.memset(zk[:], 0.0)
        myk128 = sml.tile([1, 128], dt.float32)
        nc.vector.tensor_scalar(myk128[:], zk[:], nkar[0:1, 0:1], None, Alu.add)
        kin = dram.tile([1, 128], dt.float32, tag="kin")
        kout = nc.dram_tensor("kout", [NCORES, 128], dt.float32,
                              addr_space="Shared")
        nc.sync.dma_start(kin[:], myk128[:])
        nc.gpsimd.collective_compute(
            "AllGather",
            Alu.bypass,
            replica_groups=[list(range(NCORES))],
            ins=[kin[:].opt()],
            outs=[kout[:].opt()],
        )
        kall = sml.tile([1, NCORES], dt.float32)
        nc.sync.dma_start(kall[:], kout[:, 0:1])
        gnk = sml.tile([1, 1], dt.float32)
        nc.vector.tensor_reduce(gnk[:], kall[:], AX.X, Alu.max)
        gkey = sml.tile([1, 1], dt.float32)
        nc.vector.tensor_scalar(gkey[:], gnk[:], -1.0, None, Alu.mult)
        gkey_i = sml.tile([1, 1], dt.int32)
        nc.vector.tensor_copy(gkey_i[:], gkey[:])
        seed_i = sml.tile([1, 1], dt.int32)
        nc.vector.tensor_scalar(seed_i[:], gkey_i[:], 4095, None, Alu.bitwise_and)
        seed_f = sml.tile([1, 1], dt.float32)
        nc.vector.tensor_copy(seed_f[:], seed_i[:])

        # ---- P5: sim_seed row [1, 4096] (cheap weight loads) -------------
        seed_b = sml.tile([128, 1], dt.float32)
        nc.gpsimd.partition_broadcast(seed_b[:], seed_f[:], channels=128)
        fsidxf = sml.tile([128, 1], dt.float32)
        nc.vector.tensor_scalar(fsidxf[:], t_cmult[:], seed_b[:], None, Alu.add)
        fsidx = sml.tile([128, 1], dt.int16)
        nc.vector.tensor_copy(fsidx[:], fsidxf[:])
        fs16 = sml.tile([128, 16], dt.float32)
        FT_flat = FT[:].rearrange("p a b -> p (a b)")
        nc.gpsimd.ap_gather(
            fs16[:], FT_flat, fsidx[:],
            channels=128, num_elems=NCH * N, d=1, num_idxs=16,
        )
        ss_row = sml.tile([1, N], dt.float32)
        for nt in range(NIT):
            psr = ps1.tile([1, 512], dt.float32, tag="psr")
            for c in range(NCH):
                nc.tensor.matmul(
                    psr[:],
                    fs16[:, c:c + 1],
                    FT[:, c, nt * 512:(nt + 1) * 512],
                    start=(c == 0),
                    stop=(c == NCH - 1),
                )
            nc.vector.tensor_copy(ss_row[0:1, nt * 512:(nt + 1) * 512], psr[:])
        # bounce through DRAM to relayout (DMA AP balancer limit)
        ssd = dram.tile([N], dt.float32, tag="ssd")
        nc.sync.dma_start(ssd[:], ss_row[0:1, :])
        # ss_col[p, cb] = sim_seed[cb*128 + p] for kth_largest
        ss_col = sml.tile([128, 32], dt.float32)
        nc.sync.dma_start(
            ss_col[:], ssd[:].rearrange("(c p) -> p c", p=128, c=32))
        # ss16[p16, f] = sim_seed[f*16 + p16] for compaction
        ss16 = sml.tile([16, 256], dt.float32)
        nc.sync.dma_start(
            ss16[:], ssd[:].rearrange("(f p) -> p f", p=16, f=256))

        # ---- P6: exact 100th-largest threshold ---------------------------
        kth = sml.tile([1, 2], dt.float32)
        nc.gpsimd.kth_largest(
            kth[:], ss_col[:], n_per_lane=32, k=K + 2,
            quantile=1.0 - 98.5 / 4095.0)
        Tthr = sml.tile([1, 1], dt.float32)
        nc.vector.tensor_copy(Tthr[:], kth[0:1, 1:2])

        # ---- P7: keep mask + compaction ----------------------------------
        T16 = sml.tile([16, 1], dt.float32)
        nc.gpsimd.partition_broadcast(T16[:], Tthr[:], channels=16)
        keep16 = sml.tile([16, 256], dt.float32)
        nc.vector.tensor_scalar(keep16[:], ss16[:], T16[:], None, Alu.is_ge)
        jv1 = sml.tile([16, 256], dt.float32)
        nc.gpsimd.iota(jv1[:], pattern=[[16, 256]], base=1, channel_multiplier=1,
                       allow_small_or_imprecise_dtypes=True)
        arr1 = sml.tile([16, 256], dt.float32)
        nc.vector.tensor_tensor(arr1[:], keep16[:], jv1[:], Alu.mult)
        nc.vector.tensor_scalar(arr1[:], arr1[:], -1.0, None, Alu.add)
        ge0 = sml.tile([16, 256], dt.float32)
        nc.vector.tensor_scalar(ge0[:], ss16[:], 0.0, None, Alu.is_ge)
        arr2 = sml.tile([16, 256], dt.float32)
        nc.vector.scalar_tensor_tensor(
            arr2[:], ge0[:], 2.0, keep16[:], Alu.add, Alu.mult)
        nc.vector.tensor_scalar(arr2[:], arr2[:], -1.0, None, Alu.add)
        idxf16 = sml.tile([16, 8], dt.float32)
        curf16 = sml.tile([16, 8], dt.float32)
        nf1 = sml.tile([1, 1], dt.uint32)
        nf2 = sml.tile([1, 1], dt.uint32)
        nc.gpsimd.sparse_gather(idxf16[:], arr1[:], num_found=nf1[:])
        nc.gpsimd.sparse_gather(curf16[:], arr2[:], num_found=nf2[:])
        # clean pads (l >= 100): idx -> -1, cur01 -> 0
        lg = sml.tile([16, 8], dt.int32)
        nc.gpsimd.iota(lg[:], pattern=[[16, 8]], base=0, channel_multiplier=1)
        lgf = sml.tile([16, 8], dt.float32)
        nc.vector.tensor_copy(lgf[:], lg[:])
        validl = sml.tile([16, 8], dt.float32)
        nc.vector.tensor_scalar(validl[:], lgf[:], float(K) - 0.5, None, Alu.is_le)
        idxc16 = sml.tile([16, 8], dt.float32)
        nc.vector.scalar_tensor_tensor(
            idxc16[:], idxf16[:], 1.0, validl[:], Alu.add, Alu.mult)
        nc.vector.tensor_scalar(idxc16[:], idxc16[:], -1.0, None, Alu.add)
        cur16 = sml.tile([16, 8], dt.float32)
        nc.vector.scalar_tensor_tensor(
            cur16[:], curf16[:], -1.0, validl[:], Alu.add, Alu.mult)

        # ---- P8: gather F_A^T chunks; Gram matrix M ----------------------
        idx_row = sml.tile([1, KPAD], dt.float32)
        cur_row = sml.tile([1, KPAD], dt.float32)
        for f in range(7):
            nc.sync.dma_start(idx_row[0:1, f * 16:(f + 1) * 16], idxc16[:, f:f + 1])
            nc.sync.dma_start(cur_row[0:1, f * 16:(f + 1) * 16], cur16[:, f:f + 1])
        # wrapped idx for ap_gather: replicate [16, 7] -> [128, 7]
        idx_wr = sml.tile([128, 7], dt.float32)
        for g in range(8):
            nc.sync.dma_start(idx_wr[g * 16:(g + 1) * 16, :], idxc16[:, 0:7])
        nc.vector.tensor_scalar_max(idx_wr[:], idx_wr[:], 0.0)
        FA_T = sml.tile([128, NCH, KPAD], dt.float32)
        for c in range(NCH):
            idx_wf = sml.tile([128, 7], dt.float32, tag="idxwf")
            nc.vector.tensor_scalar(
                idx_wf[:], idx_wr[:], float(c * N), None, Alu.add)
            idx_wi = sml.tile([128, 7], dt.int16, tag="idxwi")
            nc.vector.tensor_copy(idx_wi[:], idx_wf[:])
            nc.gpsimd.ap_gather(
                FA_T[:, c, :], FT_flat, idx_wi[:],
                channels=128, num_elems=NCH * N, d=1, num_idxs=KPAD,
            )
        pM = ps1.tile([KPAD, KPAD], dt.float32, tag="pM")
        for c in range(NCH):
            nc.tensor.matmul(
                pM[:], FA_T[:, c, :], FA_T[:, c, :],
                start=(c == 0), stop=(c == NCH - 1),
            )
        M_sb = sml.tile([KPAD, KPAD], dt.float32)
        nc.vector.tensor_copy(M_sb[:], pM[:])
        M_flat = big.tile([1, KPAD * KPAD], dt.float32, tag="FT")
        nc.sync.dma_start(
            M_flat[:].rearrange("o (p f) -> o p f", p=KPAD, f=KPAD), M_sb[:])

        # ---- sequential region-growing over the 100 kept positions -------
        # state[j] = new value for processed j, initial value otherwise
        state = sml.tile([1, KPAD], dt.float32)
        nc.vector.tensor_copy(state[:], cur_row[:])
        prod = sml.tile([1, KPAD], dt.float32)
        s1 = sml.tile([1, 1], dt.float32)
        for k in range(K):
            nc.vector.scalar_tensor_tensor(
                prod[:], M_flat[0:1, k * KPAD:(k + 1) * KPAD], 0.0, state[:],
                Alu.bypass, Alu.mult, accum_out=s1[:])
            nc.vector.scalar_tensor_tensor(
                state[0:1, k:k + 1], s1[:], 0.0, cur_row[0:1, k:k + 1],
                Alu.is_gt, Alu.mult)

        # ---- P9: scatter back via one-hot matmul (bf16: values are 0/1) --
        act_col = sml.tile([KPAD, 1], dt.float32)
        nc.sync.dma_start(act_col[:], state[0:1, :])
        act_bf = sml.tile([KPAD, 1], dt.bfloat16)
        nc.vector.tensor_copy(act_bf[:], act_col[:])
        idx_col = sml.tile([KPAD, 1], dt.float32)
        nc.sync.dma_start(idx_col[:], idx_row[0:1, :])
        jgrid = big.tile([128, N], dt.float32, tag="FT")
        nc.gpsimd.iota(jgrid[:], pattern=[[1, N]], base=0, channel_multiplier=0,
                       allow_small_or_imprecise_dtypes=True)
        E = big.tile([KPAD, N], dt.bfloat16, tag="Wp")
        nc.vector.tensor_scalar(
            E[:], jgrid[0:KPAD, :], idx_col[:], None, Alu.is_equal)
        pout = ps1.tile([128, 32], dt.float32, tag="psr")
        for cb in range(32):
            nc.tensor.matmul(
                pout[:, cb:cb + 1], E[:, cb * 128:(cb + 1) * 128], act_bf[:],
                start=True, stop=True)
        outcol = sml.tile([128, 32], dt.float32)
        nc.vector.tensor_copy(outcol[:], pout[:])
        nc.sync.dma_start(
            out[:].rearrange("(c p) -> p c", p=128, c=32), outcol[:])

    nc.compile()
    return nc


_NC_CACHE = None


def _get_nc():
    global _NC_CACHE
    if _NC_CACHE is None:
        _NC_CACHE = build_program()
    return _NC_CACHE


def make_in_maps(img: np.ndarray, W_patch: np.ndarray):
    img = np.asarray(img, np.float32)
    W_patch = np.asarray(W_patch, np.float32)
    x = img[0].reshape(3, 64, P, 64, P).transpose(1, 3, 2, 4, 0)
    patches = np.ascontiguousarray(x).reshape(N, P * P * 3)
    pT = np.ascontiguousarray(patches.T)          # [768, 4096]
    pT6 = pT.reshape(NCH, 128, N)
    Wp = np.ascontiguousarray(W_patch.reshape(NCH, 128, D))
    pvals = np.arange(128) % 16
    cmult = np.where(pvals < NCH, pvals * float(N), 0.0).astype(np.float32)
    cmult = cmult.reshape(128, 1)
    in_maps = []
    for c in range(NCORES):
        in_maps.append({
            "pTs": np.ascontiguousarray(pT6[:, :, c * JSL:(c + 1) * JSL]),
            "Wp": Wp,
            "jbase": np.array([[c * JSL]], np.float32),
            "cmult": cmult,
        })
    return in_maps


def kernel(img: np.ndarray, W_patch: np.ndarray) -> np.ndarray:
    nc = _get_nc()
    in_maps = make_in_maps(img, W_patch)
    res = run_bass_kernel_spmd(nc, in_maps, core_ids=list(range(NCORES)))
    return res.results[0]["out"].reshape(64, 64).astype(np.float32)


# revision 14
# speedup vs baseline: 1.1087x; 1.0989x over previous
"""Fused NMS-detection kernel for Trainium2 (8 NeuronCores, Bass/Tile).

Matches reference.py:
  patchify -> feats = patches @ W -> sim = feats @ feats.T -> degree counts
  -> seed = argmin(counts) -> top-K(sim[seed]) keep mask -> sequential
  region-growing scan -> [64, 64] binary mask.

The 4096x4096 sim matrix is never materialized to HBM: column counts of
(sim >= 0) are reduced tile-by-tile out of PSUM, and the sequential scan
runs on a compacted 112x112 Gram matrix M = F_A @ F_A^T over the <=100
kept positions (recurrence s_k = M[k, :] . state, where state holds
updated values for processed positions and initial values otherwise).

Sharding: patch columns split 512/core; each core computes its F^T slice;
6 chunked AllGathers assemble the full F^T (12.6MB) on every core; the
count phase is j-sliced per core; after the (tiny) argmin AllGather the
cheap tail phases run replicated on all cores.
"""
import sys

sys.path.insert(0, "/opt/trn_rl_repo")

from contextlib import ExitStack

import numpy as np

import concourse.bass as bass
import concourse.bacc as bacc
import concourse.bass_isa as bass_isa
import concourse.mybir as mybir
import concourse.tile as tile
from concourse.bass_utils import run_bass_kernel_spmd

dt = mybir.dt
Alu = mybir.AluOpType
AX = mybir.AxisListType

P = 16          # patch size
D = 768         # embed dim
N = 4096        # num patches (64x64)
K = 100         # top-K kept
NCORES = 8
JSL = N // NCORES   # 512 j-columns per core
NCH = D // 128      # 6 contraction chunks
KPAD = 112          # kept list padded to 16*7
NEG = -1.0e30


def build_program():
    nc = bacc.Bacc(None, "TRN2", target_bir_lowering=False, num_devices=NCORES)

    pTs = nc.dram_tensor("pTs", [NCH, 128, JSL], dt.float32, kind="ExternalInput")
    Wp = nc.dram_tensor("Wp", [NCH, 128, D], dt.float32, kind="ExternalInput")
    jbase = nc.dram_tensor("jbase", [1, 1], dt.float32, kind="ExternalInput")
    cmult = nc.dram_tensor("cmult", [128, 1], dt.float32, kind="ExternalInput")
    out = nc.dram_tensor("out", [N], dt.float32, kind="ExternalOutput")

    with tile.TileContext(nc) as tc, ExitStack() as ctx:
        big = ctx.enter_context(tc.tile_pool(name="big", bufs=1))
        sml = ctx.enter_context(tc.tile_pool(name="sml", bufs=1))
        ps = ctx.enter_context(tc.tile_pool(name="ps", bufs=4, space="PSUM"))
        ps1 = ctx.enter_context(tc.tile_pool(name="ps1", bufs=1, space="PSUM"))
        dram = ctx.enter_context(tc.tile_pool(name="dram", bufs=1, space="DRAM"))

        # ---- load inputs -------------------------------------------------
        t_pTs = big.tile([128, NCH, JSL], dt.float32, tag="pTs")
        for c in range(NCH):
            nc.sync.dma_start(t_pTs[:, c, :], pTs[c, :, :])
        t_Wp = big.tile([128, NCH, D], dt.float32, tag="Wp")
        for c in range(NCH):
            nc.sync.dma_start(t_Wp[:, c, :], Wp[c, :, :])
        t_jbase = sml.tile([1, 1], dt.float32)
        nc.sync.dma_start(t_jbase[:], jbase[:])
        t_cmult = sml.tile([128, 1], dt.float32)
        nc.sync.dma_start(t_cmult[:], cmult[:])

        # jgrid generated early, off the critical path
        jgrid = big.tile([128, N], dt.float32, tag="jgrid")
        nc.gpsimd.iota(jgrid[:], pattern=[[1, N]], base=0, channel_multiplier=0,
                       allow_small_or_imprecise_dtypes=True)

        # ---- P1: myJ = F^T[:, my j-slice]  (6 d-chunks x [128, 512]) -----
        myJ = big.tile([128, NCH, JSL], dt.float32, tag="myJ")
        for db in range(NCH):
            pacc = ps.tile([128, JSL], dt.float32, tag="pbig")
            for c in range(NCH):
                nc.tensor.matmul(
                    pacc[:],
                    t_Wp[:, c, db * 128:(db + 1) * 128],
                    t_pTs[:, c, :],
                    start=(c == 0),
                    stop=(c == NCH - 1),
                )
            nc.scalar.copy(myJ[:, db, :], pacc[:])

        # ---- P2: AllGather the full F^T (Shared outputs) -----------------
        FT = big.tile([128, NCH, N], dt.float32, tag="FT")
        agouts = [
            nc.dram_tensor(f"agout{c}", [NCORES, 128, JSL], dt.float32,
                           addr_space="Shared")
            for c in range(NCH)
        ]
        for c in range(NCH):
            agin = dram.tile([128, JSL], dt.float32, tag=f"agin{c}")
            nc.sync.dma_start(agin[:], myJ[:, c, :])
            nc.gpsimd.collective_compute(
                "AllGather",
                Alu.bypass,
                replica_groups=[list(range(NCORES))],
                ins=[agin[:].opt()],
                outs=[agouts[c][:].opt()],
            )
            # [g, p, j'] -> FT[p, c, g*512 + j']
            for g in range(NCORES):
                nc.sync.dma_start(
                    FT[:, c, g * JSL:(g + 1) * JSL], agouts[c][g, :, :])

        # ---- P3: counts of (sim >= 0) for my j-slice ---------------------
        NIT = N // 512   # 8 i-tiles
        NJB = JSL // 128  # 4 j-blocks
        acc = sml.tile([128, NJB * NIT], dt.float32)
        for jb in range(NJB):
            for it in range(NIT):
                psim = ps.tile([128, 512], dt.float32, tag="pbig")
                for c in range(NCH):
                    nc.tensor.matmul(
                        psim[:],
                        myJ[:, c, jb * 128:(jb + 1) * 128],
                        FT[:, c, it * 512:(it + 1) * 512],
                        start=(c == 0),
                        stop=(c == NCH - 1),
                    )
                nc.vector.tensor_scalar(
                    psim[:], psim[:], 0.0, None, Alu.is_ge, Alu.add,
                    accum_out=acc[:, jb * NIT + it:jb * NIT + it + 1],
                )
        counts = sml.tile([128, NJB], dt.float32)
        for jb in range(NJB):
            nc.vector.tensor_reduce(
                counts[:, jb:jb + 1], acc[:, jb * NIT:(jb + 1) * NIT], AX.X, Alu.add)

        # ---- P4: global argmin seed --------------------------------------
        # negkey = -(count*4096 + jglobal); maximize
        jb_b = sml.tile([128, 1], dt.float32)
        nc.gpsimd.partition_broadcast(jb_b[:], t_jbase[:], channels=128)
        jloc = sml.tile([128, NJB], dt.int32)
        nc.gpsimd.iota(jloc[:], pattern=[[128, NJB]], base=0, channel_multiplier=1)
        jlocf = sml.tile([128, NJB], dt.float32)
        nc.vector.tensor_copy(jlocf[:], jloc[:])
        jglob = sml.tile([128, NJB], dt.float32)
        nc.vector.tensor_scalar(jglob[:], jlocf[:], jb_b[:], None, Alu.add)
        negkey = sml.tile([128, NJB], dt.float32)
        nc.vector.scalar_tensor_tensor(
            negkey[:], counts[:], -4096.0, jglob[:], Alu.mult, Alu.subtract)
        nk1 = sml.tile([128, 1], dt.float32)
        nc.vector.tensor_reduce(nk1[:], negkey[:], AX.X, Alu.max)
        nkar = sml.tile([128, 1], dt.float32)
        nc.gpsimd.partition_all_reduce(
            nkar[:], nk1[:], channels=128, reduce_op=bass_isa.ReduceOp.max)
        # pad the collective payload to 512B
        zk = sml.tile([1, 128], dt.float32)
        nc.vector.memset(zk[:], 0.0)
        myk128 = sml.tile([1, 128], dt.float32)
        nc.vector.tensor_scalar(myk128[:], zk[:], nkar[0:1, 0:1], None, Alu.add)
        kin = dram.tile([1, 128], dt.float32, tag="kin")
        kout = nc.dram_tensor("kout", [NCORES, 128], dt.float32,
                              addr_space="Shared")
        nc.sync.dma_start(kin[:], myk128[:])
        nc.gpsimd.collective_compute(
            "AllGather",
            Alu.bypass,
            replica_groups=[list(range(NCORES))],
            ins=[kin[:].opt()],
            outs=[kout[:].opt()],
        )
        kall = sml.tile([1, NCORES], dt.float32)
        nc.sync.dma_start(kall[:], kout[:, 0:1])
        gnk = sml.tile([1, 1], dt.float32)
        nc.vector.tensor_reduce(gnk[:], kall[:], AX.X, Alu.max)
        gkey = sml.tile([1, 1], dt.float32)
        nc.vector.tensor_scalar(gkey[:], gnk[:], -1.0, None, Alu.mult)
        gkey_i = sml.tile([1, 1], dt.int32)
        nc.vector.tensor_copy(gkey_i[:], gkey[:])
        seed_i = sml.tile([1, 1], dt.int32)
        nc.vector.tensor_scalar(seed_i[:], gkey_i[:], 4095, None, Alu.bitwise_and)
        seed_f = sml.tile([1, 1], dt.float32)
        nc.vector.tensor_copy(seed_f[:], seed_i[:])

        # ---- P5: sim_seed row [1, 4096] (cheap weight loads) -------------
        seed_b = sml.tile([128, 1], dt.float32)
        nc.gpsimd.partition_broadcast(seed_b[:], seed_f[:], channels=128)
        fsidxf = sml.tile([128, 1], dt.float32)
        nc.vector.tensor_scalar(fsidxf[:], t_cmult[:], seed_b[:], None, Alu.add)
        fsidx = sml.tile([128, 1], dt.int16)
        nc.vector.tensor_copy(fsidx[:], fsidxf[:])
        fs16 = sml.tile([128, 16], dt.float32)
        FT_flat = FT[:].rearrange("p a b -> p (a b)")
        nc.gpsimd.ap_gather(
            fs16[:], FT_flat, fsidx[:],
            channels=128, num_elems=NCH * N, d=1, num_idxs=16,
        )
        ss_row = sml.tile([1, N], dt.float32)
        for nt in range(NIT):
            psr = ps1.tile([1, 512], dt.float32, tag="psr")
            for c in range(NCH):
                nc.tensor.matmul(
                    psr[:],
                    fs16[:, c:c + 1],
                    FT[:, c, nt * 512:(nt + 1) * 512],
                    start=(c == 0),
                    stop=(c == NCH - 1),
                )
            nc.vector.tensor_copy(ss_row[0:1, nt * 512:(nt + 1) * 512], psr[:])
        # bounce through DRAM to relayout (DMA AP balancer limit)
        ssd = dram.tile([N], dt.float32, tag="ssd")
        nc.sync.dma_start(ssd[:], ss_row[0:1, :])
        # ss_col[p, cb] = sim_seed[cb*128 + p] for kth_largest
        ss_col = sml.tile([128, 32], dt.float32)
        nc.sync.dma_start(
            ss_col[:], ssd[:].rearrange("(c p) -> p c", p=128, c=32))
        # ss16[p16, f] = sim_seed[f*16 + p16] for compaction
        ss16 = sml.tile([16, 256], dt.float32)
        nc.sync.dma_start(
            ss16[:], ssd[:].rearrange("(f p) -> p f", p=16, f=256))

        # ---- P6: top-100 threshold via max8/match_replace cascade --------
        r1 = sml.tile([128, 8], dt.float32)
        nc.vector.max(r1[:], ss_col[:])
        ssr = sml.tile([128, 32], dt.float32)
        nc.vector.match_replace(ssr[:], r1[:], ss_col[:], NEG)
        cand = sml.tile([128, 16], dt.float32)
        nc.vector.tensor_copy(cand[:, 0:8], r1[:])
        nc.vector.max(cand[:, 8:16], ssr[:])
        # regroup to [16, 128]: cgrp[p16, g*16+r] = cand[g*16+p16, r]
        cgrp = sml.tile([16, 128], dt.float32)
        for g in range(8):
            nc.sync.dma_start(
                cgrp[:, g * 16:(g + 1) * 16], cand[g * 16:(g + 1) * 16, :])
        pool24 = sml.tile([16, 24], dt.float32)
        nc.vector.max(pool24[:, 0:8], cgrp[:])
        cg2 = sml.tile([16, 128], dt.float32)
        nc.vector.match_replace(cg2[:], pool24[:, 0:8], cgrp[:], NEG)
        nc.vector.max(pool24[:, 8:16], cg2[:])
        cg3 = sml.tile([16, 128], dt.float32)
        nc.vector.match_replace(cg3[:], pool24[:, 8:16], cg2[:], NEG)
        nc.vector.max(pool24[:, 16:24], cg3[:])
        wrk = sml.tile([1, 384], dt.float32, tag="wrkA")
        nc.sync.dma_start(
            wrk[:].rearrange("o (p f) -> o p f", p=16, f=24), pool24[:])
        m8 = sml.tile([1, 8], dt.float32)
        for r in range(13):
            nc.vector.max(m8[:], wrk[:])
            if r < 12:
                wrk2 = sml.tile([1, 384], dt.float32,
                                tag=f"wrk{'B' if r % 2 == 0 else 'A'}")
                nc.vector.match_replace(wrk2[:], m8[:], wrk[:], NEG)
                wrk = wrk2
        Tthr = sml.tile([1, 1], dt.float32)
        nc.vector.tensor_copy(Tthr[:], m8[:, 3:4])

        # ---- P7: keep mask + compaction ----------------------------------
        T16 = sml.tile([16, 1], dt.float32)
        nc.gpsimd.partition_broadcast(T16[:], Tthr[:], channels=16)
        keep16 = sml.tile([16, 256], dt.float32)
        nc.vector.tensor_scalar(keep16[:], ss16[:], T16[:], None, Alu.is_ge)
        jv1 = sml.tile([16, 256], dt.float32)
        nc.gpsimd.iota(jv1[:], pattern=[[16, 256]], base=1, channel_multiplier=1,
                       allow_small_or_imprecise_dtypes=True)
        arr1 = sml.tile([16, 256], dt.float32)
        nc.vector.tensor_tensor(arr1[:], keep16[:], jv1[:], Alu.mult)
        nc.vector.tensor_scalar(arr1[:], arr1[:], -1.0, None, Alu.add)
        ge0 = sml.tile([16, 256], dt.float32)
        nc.vector.tensor_scalar(ge0[:], ss16[:], 0.0, None, Alu.is_ge)
        arr2 = sml.tile([16, 256], dt.float32)
        nc.vector.scalar_tensor_tensor(
            arr2[:], ge0[:], 2.0, keep16[:], Alu.add, Alu.mult)
        nc.vector.tensor_scalar(arr2[:], arr2[:], -1.0, None, Alu.add)
        idxf16 = sml.tile([16, 8], dt.float32)
        curf16 = sml.tile([16, 8], dt.float32)
        nf1 = sml.tile([1, 1], dt.uint32)
        nf2 = sml.tile([1, 1], dt.uint32)
        nc.gpsimd.sparse_gather(idxf16[:], arr1[:], num_found=nf1[:])
        nc.gpsimd.sparse_gather(curf16[:], arr2[:], num_found=nf2[:])
        # clean pads (l >= 100): idx -> -1, cur01 -> 0
        lg = sml.tile([16, 8], dt.int32)
        nc.gpsimd.iota(lg[:], pattern=[[16, 8]], base=0, channel_multiplier=1)
        lgf = sml.tile([16, 8], dt.float32)
        nc.vector.tensor_copy(lgf[:], lg[:])
        validl = sml.tile([16, 8], dt.float32)
        nc.vector.tensor_scalar(validl[:], lgf[:], float(K) - 0.5, None, Alu.is_le)
        idxc16 = sml.tile([16, 8], dt.float32)
        nc.vector.scalar_tensor_tensor(
            idxc16[:], idxf16[:], 1.0, validl[:], Alu.add, Alu.mult)
        nc.vector.tensor_scalar(idxc16[:], idxc16[:], -1.0, None, Alu.add)
        cur16 = sml.tile([16, 8], dt.float32)
        nc.vector.scalar_tensor_tensor(
            cur16[:], curf16[:], -1.0, validl[:], Alu.add, Alu.mult)

        # ---- P8: gather F_A^T chunks; Gram matrix M ----------------------
        idx_row = sml.tile([1, KPAD], dt.float32)
        cur_row = sml.tile([1, KPAD], dt.float32)
        for f in range(7):
            nc.sync.dma_start(idx_row[0:1, f * 16:(f + 1) * 16], idxc16[:, f:f + 1])
            nc.sync.dma_start(cur_row[0:1, f * 16:(f + 1) * 16], cur16[:, f:f + 1])
        # wrapped idx for ap_gather: replicate [16, 7] -> [128, 7]
        idx_wr = sml.tile([128, 7], dt.float32)
        for g in range(8):
            nc.sync.dma_start(idx_wr[g * 16:(g + 1) * 16, :], idxc16[:, 0:7])
        nc.vector.tensor_scalar_max(idx_wr[:], idx_wr[:], 0.0)
        FA_T = sml.tile([128, NCH, KPAD], dt.float32)
        for c in range(NCH):
            idx_wf = sml.tile([128, 7], dt.float32, tag="idxwf")
            nc.vector.tensor_scalar(
                idx_wf[:], idx_wr[:], float(c * N), None, Alu.add)
            idx_wi = sml.tile([128, 7], dt.int16, tag="idxwi")
            nc.vector.tensor_copy(idx_wi[:], idx_wf[:])
            nc.gpsimd.ap_gather(
                FA_T[:, c, :], FT_flat, idx_wi[:],
                channels=128, num_elems=NCH * N, d=1, num_idxs=KPAD,
            )
        pM = ps1.tile([KPAD, KPAD], dt.float32, tag="pM")
        for c in range(NCH):
            nc.tensor.matmul(
                pM[:], FA_T[:, c, :], FA_T[:, c, :],
                start=(c == 0), stop=(c == NCH - 1),
            )
        M_sb = sml.tile([KPAD, KPAD], dt.float32)
        nc.vector.tensor_copy(M_sb[:], pM[:])
        M_flat = big.tile([1, KPAD * KPAD], dt.float32, tag="FT")
        nc.sync.dma_start(
            M_flat[:].rearrange("o (p f) -> o p f", p=KPAD, f=KPAD), M_sb[:])

        # ---- sequential region-growing over the 100 kept positions -------
        # state[j] = new value for processed j, initial value otherwise
        state = sml.tile([1, KPAD], dt.float32)
        nc.vector.tensor_copy(state[:], cur_row[:])
        prod = sml.tile([1, KPAD], dt.float32)
        s1 = sml.tile([1, 1], dt.float32)
        for k in range(K):
            nc.vector.scalar_tensor_tensor(
                prod[:], M_flat[0:1, k * KPAD:(k + 1) * KPAD], 0.0, state[:],
                Alu.bypass, Alu.mult, accum_out=s1[:])
            nc.vector.scalar_tensor_tensor(
                state[0:1, k:k + 1], s1[:], 0.0, cur_row[0:1, k:k + 1],
                Alu.is_gt, Alu.mult)

        # ---- P9: scatter back via one-hot matmul (bf16: values are 0/1) --
        act_col = sml.tile([KPAD, 1], dt.float32)
        nc.sync.dma_start(act_col[:], state[0:1, :])
        act_bf = sml.tile([KPAD, 1], dt.bfloat16)
        nc.vector.tensor_copy(act_bf[:], act_col[:])
        idx_col = sml.tile([KPAD, 1], dt.float32)
        nc.sync.dma_start(idx_col[:], idx_row[0:1, :])
        E = big.tile([KPAD, N], dt.bfloat16, tag="Wp")
        nc.vector.tensor_scalar(
            E[:], jgrid[0:KPAD, :], idx_col[:], None, Alu.is_equal)
        pout = ps1.tile([128, 32], dt.float32, tag="psr")
        for cb in range(32):
            nc.tensor.matmul(
                pout[:, cb:cb + 1], E[:, cb * 128:(cb + 1) * 128], act_bf[:],
                start=True, stop=True)
        outcol = sml.tile([128, 32], dt.float32)
        nc.vector.tensor_copy(outcol[:], pout[:])
        nc.sync.dma_start(
            out[:].rearrange("(c p) -> p c", p=128, c=32), outcol[:])

    nc.compile()
    return nc


_NC_CACHE = None


def _get_nc():
    global _NC_CACHE
    if _NC_CACHE is None:
        _NC_CACHE = build_program()
    return _NC_CACHE


def make_in_maps(img: np.ndarray, W_patch: np.ndarray):
    img = np.asarray(img, np.float32)
    W_patch = np.asarray(W_patch, np.float32)
    x = img[0].reshape(3, 64, P, 64, P).transpose(1, 3, 2, 4, 0)
    patches = np.ascontiguousarray(x).reshape(N, P * P * 3)
    pT = np.ascontiguousarray(patches.T)          # [768, 4096]
    pT6 = pT.reshape(NCH, 128, N)
    Wp = np.ascontiguousarray(W_patch.reshape(NCH, 128, D))
    pvals = np.arange(128) % 16
    cmult = np.where(pvals < NCH, pvals * float(N), 0.0).astype(np.float32)
    cmult = cmult.reshape(128, 1)
    in_maps = []
    for c in range(NCORES):
        in_maps.append({
            "pTs": np.ascontiguousarray(pT6[:, :, c * JSL:(c + 1) * JSL]),
            "Wp": Wp,
            "jbase": np.array([[c * JSL]], np.float32),
            "cmult": cmult,
        })
    return in_maps


def kernel(img: np.ndarray, W_patch: np.ndarray) -> np.ndarray:
    nc = _get_nc()
    in_maps = make_in_maps(img, W_patch)
    res = run_bass_kernel_spmd(nc, in_maps, core_ids=list(range(NCORES)))
    return res.results[0]["out"].reshape(64, 64).astype(np.float32)


# revision 18
# speedup vs baseline: 1.1958x; 1.0786x over previous
"""Fused NMS-detection kernel for Trainium2 (8 NeuronCores, Bass/Tile).

Matches reference.py:
  patchify -> feats = patches @ W -> sim = feats @ feats.T -> degree counts
  -> seed = argmin(counts) -> top-K(sim[seed]) keep mask -> sequential
  region-growing scan -> [64, 64] binary mask.

The 4096x4096 sim matrix is never materialized to HBM: column counts of
(sim >= 0) are reduced tile-by-tile out of PSUM, and the sequential scan
runs on a compacted 112x112 Gram matrix M = F_A @ F_A^T over the <=100
kept positions (recurrence s_k = M[k, :] . state, where state holds
updated values for processed positions and initial values otherwise).

Sharding: patch columns split 512/core; each core computes its F^T slice;
6 chunked AllGathers assemble the full F^T (12.6MB) on every core; the
count phase is j-sliced per core; after the (tiny) argmin AllGather the
cheap tail phases run replicated on all cores.
"""
import sys

sys.path.insert(0, "/opt/trn_rl_repo")

from contextlib import ExitStack

import numpy as np

import concourse.bass as bass
import concourse.bacc as bacc
import concourse.bass_isa as bass_isa
import concourse.mybir as mybir
import concourse.tile as tile
from concourse.bass_utils import run_bass_kernel_spmd
import concourse.bass_utils as _bu

if not getattr(_bu, "_ldw_opt_patched", False):
    _orig_run_command = _bu.run_command

    def _run_command_ldw(argv, **kw):
        return _orig_run_command(argv, **kw)

    _bu.run_command = _run_command_ldw
    _bu._ldw_opt_patched = True

dt = mybir.dt
Alu = mybir.AluOpType
AX = mybir.AxisListType

P = 16          # patch size
D = 768         # embed dim
N = 4096        # num patches (64x64)
K = 100         # top-K kept
NCORES = 8
JSL = N // NCORES   # 512 j-columns per core
NCH = D // 128      # 6 contraction chunks
KPAD = 112          # kept list padded to 16*7
NEG = -1.0e30


def build_program():
    nc = bacc.Bacc(None, "TRN2", target_bir_lowering=False, num_devices=NCORES)

    pTs = nc.dram_tensor("pTs", [NCH, 128, JSL], dt.float32, kind="ExternalInput")
    Wp = nc.dram_tensor("Wp", [NCH, 128, D], dt.float32, kind="ExternalInput")
    jbase = nc.dram_tensor("jbase", [1, 1], dt.float32, kind="ExternalInput")
    cmult = nc.dram_tensor("cmult", [128, 1], dt.float32, kind="ExternalInput")
    out = nc.dram_tensor("out", [N], dt.float32, kind="ExternalOutput")

    with tile.TileContext(nc) as tc, ExitStack() as ctx:
        big = ctx.enter_context(tc.tile_pool(name="big", bufs=1))
        sml = ctx.enter_context(tc.tile_pool(name="sml", bufs=1))
        ps = ctx.enter_context(tc.tile_pool(name="ps", bufs=4, space="PSUM"))
        ps1 = ctx.enter_context(tc.tile_pool(name="ps1", bufs=1, space="PSUM"))
        dram = ctx.enter_context(tc.tile_pool(name="dram", bufs=1, space="DRAM"))

        # ---- load inputs -------------------------------------------------
        t_pTs = big.tile([128, NCH, JSL], dt.float32, tag="pTs")
        for c in range(NCH):
            nc.sync.dma_start(t_pTs[:, c, :], pTs[c, :, :])
        t_Wp = big.tile([128, NCH, D], dt.float32, tag="Wp")
        for c in range(NCH):
            nc.sync.dma_start(t_Wp[:, c, :], Wp[c, :, :])
        t_jbase = sml.tile([1, 1], dt.float32)
        nc.sync.dma_start(t_jbase[:], jbase[:])
        t_cmult = sml.tile([128, 1], dt.float32)
        nc.sync.dma_start(t_cmult[:], cmult[:])

        # dummy collective to absorb first-collective setup latency
        wdin = dram.tile([1, 128], dt.float32, tag="wdin")
        wdout = nc.dram_tensor("wdout", [NCORES, 128], dt.float32,
                               addr_space="Shared")
        zw = sml.tile([1, 128], dt.float32)
        nc.vector.memset(zw[:], 0.0)
        nc.sync.dma_start(wdin[:], zw[:])
        nc.gpsimd.collective_compute(
            "AllGather",
            Alu.bypass,
            replica_groups=[list(range(NCORES))],
            ins=[wdin[:].opt()],
            outs=[wdout[:].opt()],
        )

        # jgrid generated early, off the critical path
        jgrid = big.tile([128, N], dt.float32, tag="jgrid")
        nc.gpsimd.iota(jgrid[:], pattern=[[1, N]], base=0, channel_multiplier=0,
                       allow_small_or_imprecise_dtypes=True)

        # ---- P1: myJ = F^T[:, my j-slice]  (6 d-chunks x [128, 512]) -----
        myJ = big.tile([128, NCH, JSL], dt.float32, tag="myJ")
        for db in range(NCH):
            pacc = ps.tile([128, JSL], dt.float32, tag="pbig")
            for c in range(NCH):
                nc.tensor.matmul(
                    pacc[:],
                    t_Wp[:, c, db * 128:(db + 1) * 128],
                    t_pTs[:, c, :],
                    start=(c == 0),
                    stop=(c == NCH - 1),
                )
            nc.scalar.copy(myJ[:, db, :], pacc[:])

        # ---- P2: AllGather the full F^T (Shared outputs) -----------------
        FT = big.tile([128, NCH, N], dt.float32, tag="FT")
        agouts = [
            nc.dram_tensor(f"agout{c}", [NCORES, 128, JSL], dt.float32,
                           addr_space="Shared")
            for c in range(NCH)
        ]
        for c in range(NCH):
            agin = dram.tile([128, JSL], dt.float32, tag=f"agin{c}")
            nc.sync.dma_start(agin[:], myJ[:, c, :])
            nc.gpsimd.collective_compute(
                "AllGather",
                Alu.bypass,
                replica_groups=[list(range(NCORES))],
                ins=[agin[:].opt()],
                outs=[agouts[c][:].opt()],
            )
            # [g, p, j'] -> FT[p, c, g*512 + j']
            for g in range(NCORES):
                nc.sync.dma_start(
                    FT[:, c, g * JSL:(g + 1) * JSL], agouts[c][g, :, :])

        # ---- P3: counts of (sim >= 0) for my j-slice ---------------------
        NIT = N // 512   # 8 i-tiles
        NJB = JSL // 128  # 4 j-blocks
        acc = sml.tile([128, NJB * NIT], dt.float32)
        stashT = nc.dram_tensor("stash", [NJB, NIT, 128, 512], dt.float32)
        # half 1: chunks 0-2 accumulate, stash to DRAM (frees PSUM during AGs)
        for jb in range(NJB):
            for it in range(NIT):
                psim = ps.tile([128, 512], dt.float32, tag="pbig",
                               name=f"psA{jb}_{it}")
                for c in range(3):
                    nc.tensor.matmul(
                        psim[:],
                        myJ[:, c, jb * 128:(jb + 1) * 128],
                        FT[:, c, it * 512:(it + 1) * 512],
                        start=(c == 0),
                        stop=(c == 2),
                    )
                stg = sml.tile([128, 512], dt.float32, tag="stg", bufs=3,
                               name=f"stg{jb}_{it}")
                nc.scalar.copy(stg[:], psim[:])
                nc.sync.dma_start(stashT[jb, it, :, :], stg[:])
        # half 2: chunks 3-5, reload stash, combine, sign-count
        for jb in range(NJB):
            for it in range(NIT):
                psim = ps.tile([128, 512], dt.float32, tag="pbig",
                               name=f"psB{jb}_{it}")
                for c in range(3, NCH):
                    nc.tensor.matmul(
                        psim[:],
                        myJ[:, c, jb * 128:(jb + 1) * 128],
                        FT[:, c, it * 512:(it + 1) * 512],
                        start=(c == 3),
                        stop=(c == NCH - 1),
                    )
                stg2 = sml.tile([128, 512], dt.float32, tag="stg2", bufs=3,
                                name=f"stg2{jb}_{it}")
                nc.sync.dma_start(stg2[:], stashT[jb, it, :, :])
                nc.vector.tensor_tensor(psim[:], psim[:], stg2[:], Alu.add)
                nc.vector.tensor_scalar(
                    psim[:], psim[:], 0.0, None, Alu.is_ge, Alu.add,
                    accum_out=acc[:, jb * NIT + it:jb * NIT + it + 1],
                )
        counts = sml.tile([128, NJB], dt.float32)
        for jb in range(NJB):
            nc.vector.tensor_reduce(
                counts[:, jb:jb + 1], acc[:, jb * NIT:(jb + 1) * NIT], AX.X, Alu.add)

        # ---- P4: global argmin seed --------------------------------------
        # negkey = -(count*4096 + jglobal); maximize
        jb_b = sml.tile([128, 1], dt.float32)
        nc.gpsimd.partition_broadcast(jb_b[:], t_jbase[:], channels=128)
        jloc = sml.tile([128, NJB], dt.int32)
        nc.gpsimd.iota(jloc[:], pattern=[[128, NJB]], base=0, channel_multiplier=1)
        jlocf = sml.tile([128, NJB], dt.float32)
        nc.vector.tensor_copy(jlocf[:], jloc[:])
        jglob = sml.tile([128, NJB], dt.float32)
        nc.vector.tensor_scalar(jglob[:], jlocf[:], jb_b[:], None, Alu.add)
        negkey = sml.tile([128, NJB], dt.float32)
        nc.vector.scalar_tensor_tensor(
            negkey[:], counts[:], -4096.0, jglob[:], Alu.mult, Alu.subtract)
        nk1 = sml.tile([128, 1], dt.float32)
        nc.vector.tensor_reduce(nk1[:], negkey[:], AX.X, Alu.max)
        nkar = sml.tile([128, 1], dt.float32)
        nc.gpsimd.partition_all_reduce(
            nkar[:], nk1[:], channels=128, reduce_op=bass_isa.ReduceOp.max)
        # pad the collective payload to 512B
        zk = sml.tile([1, 128], dt.float32)
        nc.vector.memset(zk[:], 0.0)
        myk128 = sml.tile([1, 128], dt.float32)
        nc.vector.tensor_scalar(myk128[:], zk[:], nkar[0:1, 0:1], None, Alu.add)
        kin = dram.tile([1, 128], dt.float32, tag="kin")
        kout = nc.dram_tensor("kout", [NCORES, 128], dt.float32,
                              addr_space="Shared")
        nc.sync.dma_start(kin[:], myk128[:])
        nc.gpsimd.collective_compute(
            "AllGather",
            Alu.bypass,
            replica_groups=[list(range(NCORES))],
            ins=[kin[:].opt()],
            outs=[kout[:].opt()],
        )
        kall = sml.tile([1, NCORES], dt.float32)
        nc.sync.dma_start(kall[:], kout[:, 0:1])
        gnk = sml.tile([1, 1], dt.float32)
        nc.vector.tensor_reduce(gnk[:], kall[:], AX.X, Alu.max)
        gkey = sml.tile([1, 1], dt.float32)
        nc.vector.tensor_scalar(gkey[:], gnk[:], -1.0, None, Alu.mult)
        gkey_i = sml.tile([1, 1], dt.int32)
        nc.vector.tensor_copy(gkey_i[:], gkey[:])
        seed_i = sml.tile([1, 1], dt.int32)
        nc.vector.tensor_scalar(seed_i[:], gkey_i[:], 4095, None, Alu.bitwise_and)
        seed_f = sml.tile([1, 1], dt.float32)
        nc.vector.tensor_copy(seed_f[:], seed_i[:])

        # ---- P5: sim_seed row [1, 4096] (cheap weight loads) -------------
        seed_b = sml.tile([128, 1], dt.float32)
        nc.gpsimd.partition_broadcast(seed_b[:], seed_f[:], channels=128)
        fsidxf = sml.tile([128, 1], dt.float32)
        nc.vector.tensor_scalar(fsidxf[:], t_cmult[:], seed_b[:], None, Alu.add)
        fsidx = sml.tile([128, 1], dt.int16)
        nc.vector.tensor_copy(fsidx[:], fsidxf[:])
        fs16 = sml.tile([128, 16], dt.float32)
        FT_flat = FT[:].rearrange("p a b -> p (a b)")
        nc.gpsimd.ap_gather(
            fs16[:], FT_flat, fsidx[:],
            channels=128, num_elems=NCH * N, d=1, num_idxs=16,
        )
        ss_row = sml.tile([1, N], dt.float32)
        for nt in range(NIT):
            psr = ps1.tile([1, 512], dt.float32, tag="psr")
            for c in range(NCH):
                nc.tensor.matmul(
                    psr[:],
                    fs16[:, c:c + 1],
                    FT[:, c, nt * 512:(nt + 1) * 512],
                    start=(c == 0),
                    stop=(c == NCH - 1),
                )
            nc.vector.tensor_copy(ss_row[0:1, nt * 512:(nt + 1) * 512], psr[:])
        # bounce through DRAM to relayout (DMA AP balancer limit)
        ssd = dram.tile([N], dt.float32, tag="ssd")
        nc.sync.dma_start(ssd[:], ss_row[0:1, :])
        # ss_col[p, cb] = sim_seed[cb*128 + p] for kth_largest
        ss_col = sml.tile([128, 32], dt.float32)
        nc.sync.dma_start(
            ss_col[:], ssd[:].rearrange("(c p) -> p c", p=128, c=32))
        # ss16[p16, f] = sim_seed[f*16 + p16] for compaction
        ss16 = sml.tile([16, 256], dt.float32)
        nc.sync.dma_start(
            ss16[:], ssd[:].rearrange("(f p) -> p f", p=16, f=256))

        # ---- P6: top-100 threshold via max8/match_replace cascade --------
        r1 = sml.tile([128, 8], dt.float32)
        nc.vector.max(r1[:], ss_col[:])
        ssr = sml.tile([128, 32], dt.float32)
        nc.vector.match_replace(ssr[:], r1[:], ss_col[:], NEG)
        cand = sml.tile([128, 16], dt.float32)
        nc.vector.tensor_copy(cand[:, 0:8], r1[:])
        nc.vector.max(cand[:, 8:16], ssr[:])
        # regroup to [16, 128]: cgrp[p16, g*16+r] = cand[g*16+p16, r]
        cgrp = sml.tile([16, 128], dt.float32)
        for g in range(8):
            nc.sync.dma_start(
                cgrp[:, g * 16:(g + 1) * 16], cand[g * 16:(g + 1) * 16, :])
        pool24 = sml.tile([16, 24], dt.float32)
        nc.vector.max(pool24[:, 0:8], cgrp[:])
        cg2 = sml.tile([16, 128], dt.float32)
        nc.vector.match_replace(cg2[:], pool24[:, 0:8], cgrp[:], NEG)
        nc.vector.max(pool24[:, 8:16], cg2[:])
        cg3 = sml.tile([16, 128], dt.float32)
        nc.vector.match_replace(cg3[:], pool24[:, 8:16], cg2[:], NEG)
        nc.vector.max(pool24[:, 16:24], cg3[:])
        wrk = sml.tile([1, 384], dt.float32, tag="wrkA")
        nc.sync.dma_start(
            wrk[:].rearrange("o (p f) -> o p f", p=16, f=24), pool24[:])
        m8 = sml.tile([1, 8], dt.float32)
        for r in range(13):
            nc.vector.max(m8[:], wrk[:])
            if r < 12:
                wrk2 = sml.tile([1, 384], dt.float32,
                                tag=f"wrk{'B' if r % 2 == 0 else 'A'}")
                nc.vector.match_replace(wrk2[:], m8[:], wrk[:], NEG)
                wrk = wrk2
        Tthr = sml.tile([1, 1], dt.float32)
        nc.vector.tensor_copy(Tthr[:], m8[:, 3:4])

        # ---- P7: keep mask + compaction ----------------------------------
        T16 = sml.tile([16, 1], dt.float32)
        nc.gpsimd.partition_broadcast(T16[:], Tthr[:], channels=16)
        keep16 = sml.tile([16, 256], dt.float32)
        nc.vector.tensor_scalar(keep16[:], ss16[:], T16[:], None, Alu.is_ge)
        jv1 = sml.tile([16, 256], dt.float32)
        nc.gpsimd.iota(jv1[:], pattern=[[16, 256]], base=1, channel_multiplier=1,
                       allow_small_or_imprecise_dtypes=True)
        arr1 = sml.tile([16, 256], dt.float32)
        nc.vector.tensor_tensor(arr1[:], keep16[:], jv1[:], Alu.mult)
        nc.vector.tensor_scalar(arr1[:], arr1[:], -1.0, None, Alu.add)
        ge0 = sml.tile([16, 256], dt.float32)
        nc.vector.tensor_scalar(ge0[:], ss16[:], 0.0, None, Alu.is_ge)
        arr2 = sml.tile([16, 256], dt.float32)
        nc.vector.scalar_tensor_tensor(
            arr2[:], ge0[:], 2.0, keep16[:], Alu.add, Alu.mult)
        nc.vector.tensor_scalar(arr2[:], arr2[:], -1.0, None, Alu.add)
        idxf16 = sml.tile([16, 8], dt.float32)
        curf16 = sml.tile([16, 8], dt.float32)
        nf1 = sml.tile([1, 1], dt.uint32)
        nf2 = sml.tile([1, 1], dt.uint32)
        nc.gpsimd.sparse_gather(idxf16[:], arr1[:], num_found=nf1[:])
        nc.gpsimd.sparse_gather(curf16[:], arr2[:], num_found=nf2[:])
        # clean pads (l >= 100): idx -> -1, cur01 -> 0
        lg = sml.tile([16, 8], dt.int32)
        nc.gpsimd.iota(lg[:], pattern=[[16, 8]], base=0, channel_multiplier=1)
        lgf = sml.tile([16, 8], dt.float32)
        nc.vector.tensor_copy(lgf[:], lg[:])
        validl = sml.tile([16, 8], dt.float32)
        nc.vector.tensor_scalar(validl[:], lgf[:], float(K) - 0.5, None, Alu.is_le)
        idxc16 = sml.tile([16, 8], dt.float32)
        nc.vector.scalar_tensor_tensor(
            idxc16[:], idxf16[:], 1.0, validl[:], Alu.add, Alu.mult)
        nc.vector.tensor_scalar(idxc16[:], idxc16[:], -1.0, None, Alu.add)
        cur16 = sml.tile([16, 8], dt.float32)
        nc.vector.scalar_tensor_tensor(
            cur16[:], curf16[:], -1.0, validl[:], Alu.add, Alu.mult)

        # ---- P8: gather F_A^T chunks; Gram matrix M ----------------------
        idx_row = sml.tile([1, KPAD], dt.float32)
        cur_row = sml.tile([1, KPAD], dt.float32)
        for f in range(7):
            nc.sync.dma_start(idx_row[0:1, f * 16:(f + 1) * 16], idxc16[:, f:f + 1])
            nc.sync.dma_start(cur_row[0:1, f * 16:(f + 1) * 16], cur16[:, f:f + 1])
        # wrapped idx for ap_gather: replicate [16, 7] -> [128, 7]
        idx_wr = sml.tile([128, 7], dt.float32)
        for g in range(8):
            nc.sync.dma_start(idx_wr[g * 16:(g + 1) * 16, :], idxc16[:, 0:7])
        nc.vector.tensor_scalar_max(idx_wr[:], idx_wr[:], 0.0)
        FA_T = sml.tile([128, NCH, KPAD], dt.float32)
        for c in range(NCH):
            idx_wf = sml.tile([128, 7], dt.float32, tag="idxwf")
            nc.vector.tensor_scalar(
                idx_wf[:], idx_wr[:], float(c * N), None, Alu.add)
            idx_wi = sml.tile([128, 7], dt.int16, tag="idxwi")
            nc.vector.tensor_copy(idx_wi[:], idx_wf[:])
            nc.gpsimd.ap_gather(
                FA_T[:, c, :], FT_flat, idx_wi[:],
                channels=128, num_elems=NCH * N, d=1, num_idxs=KPAD,
            )
        pM = ps1.tile([KPAD, KPAD], dt.float32, tag="pM")
        for c in range(NCH):
            nc.tensor.matmul(
                pM[:], FA_T[:, c, :], FA_T[:, c, :],
                start=(c == 0), stop=(c == NCH - 1),
            )
        M_sb = sml.tile([KPAD, KPAD], dt.float32)
        nc.vector.tensor_copy(M_sb[:], pM[:])
        M_flat = big.tile([1, KPAD * KPAD], dt.float32, tag="FT")
        nc.sync.dma_start(
            M_flat[:].rearrange("o (p f) -> o p f", p=KPAD, f=KPAD), M_sb[:])

        # ---- sequential region-growing over the 100 kept positions -------
        # state[j] = new value for processed j, initial value otherwise
        state = sml.tile([1, KPAD], dt.float32)
        nc.vector.tensor_copy(state[:], cur_row[:])
        prod = sml.tile([1, KPAD], dt.float32)
        s1 = sml.tile([1, 1], dt.float32)
        for k in range(K):
            nc.vector.scalar_tensor_tensor(
                prod[:], M_flat[0:1, k * KPAD:(k + 1) * KPAD], 0.0, state[:],
                Alu.bypass, Alu.mult, accum_out=s1[:])
            nc.vector.scalar_tensor_tensor(
                state[0:1, k:k + 1], s1[:], 0.0, cur_row[0:1, k:k + 1],
                Alu.is_gt, Alu.mult)

        # ---- P9: scatter back via one-hot matmul (bf16: values are 0/1) --
        act_col = sml.tile([KPAD, 1], dt.float32)
        nc.sync.dma_start(act_col[:], state[0:1, :])
        act_bf = sml.tile([KPAD, 1], dt.bfloat16)
        nc.vector.tensor_copy(act_bf[:], act_col[:])
        idx_col = sml.tile([KPAD, 1], dt.float32)
        nc.sync.dma_start(idx_col[:], idx_row[0:1, :])
        E = big.tile([KPAD, N], dt.bfloat16, tag="Wp")
        nc.vector.tensor_scalar(
            E[:], jgrid[0:KPAD, :], idx_col[:], None, Alu.is_equal)
        pout = ps1.tile([128, 32], dt.float32, tag="psr")
        for cb in range(32):
            nc.tensor.matmul(
                pout[:, cb:cb + 1], E[:, cb * 128:(cb + 1) * 128], act_bf[:],
                start=True, stop=True)
        outcol = sml.tile([128, 32], dt.float32)
        nc.vector.tensor_copy(outcol[:], pout[:])
        nc.sync.dma_start(
            out[:].rearrange("(c p) -> p c", p=128, c=32), outcol[:])

    nc.compile()
    return nc


_NC_CACHE = None


def _get_nc():
    global _NC_CACHE
    if _NC_CACHE is None:
        _NC_CACHE = build_program()
    return _NC_CACHE


def make_in_maps(img: np.ndarray, W_patch: np.ndarray):
    img = np.asarray(img, np.float32)
    W_patch = np.asarray(W_patch, np.float32)
    x = img[0].reshape(3, 64, P, 64, P).transpose(1, 3, 2, 4, 0)
    patches = np.ascontiguousarray(x).reshape(N, P * P * 3)
    pT = np.ascontiguousarray(patches.T)          # [768, 4096]
    pT6 = pT.reshape(NCH, 128, N)
    Wp = np.ascontiguousarray(W_patch.reshape(NCH, 128, D))
    pvals = np.arange(128) % 16
    cmult = np.where(pvals < NCH, pvals * float(N), 0.0).astype(np.float32)
    cmult = cmult.reshape(128, 1)
    in_maps = []
    for c in range(NCORES):
        in_maps.append({
            "pTs": np.ascontiguousarray(pT6[:, :, c * JSL:(c + 1) * JSL]),
            "Wp": Wp,
            "jbase": np.array([[c * JSL]], np.float32),
            "cmult": cmult,
        })
    return in_maps


def kernel(img: np.ndarray, W_patch: np.ndarray) -> np.ndarray:
    nc = _get_nc()
    in_maps = make_in_maps(img, W_patch)
    res = run_bass_kernel_spmd(nc, in_maps, core_ids=list(range(NCORES)))
    return res.results[0]["out"].reshape(64, 64).astype(np.float32)
